# revision 38
# baseline (speedup 1.0000x reference)
"""Masked dot-product attention on 8 Trainium2 NeuronCores.

Strategy (per core): head-parallel sharding. B*H = 64 (batch, head) pairs are
split 8 per core; each core runs the full attention for its heads.

All layout transforms happen on the HOST (numpy) so the device only issues
plain contiguous DMAs:
  qT/kT:  [npairs, 4, 128, 512] bf16, head i of a pair on partitions
          64i..64i+63, DK-major, chunked along S so compute can start as soon
          as the first 128KB chunk lands.
  v1:     [nheads, 128, CH, 65] bf16, kj-within-chunk on partitions, with the
          ones column baked in (row dv=64 accumulates softmax denominators).
  maskT:  [128, n_kj, S] bf16 keep-mask (1-mask), kj-within-tile on
          partitions.

Per-head-pair pipeline (S=2048, DK=64), "S-transposed" layout so the PV
matmul needs no transpose of the huge exp matrix:
  S_T[kj, qi] = K @ Q^T        (PE, bf16, psum strips [128 kj, 2x512 qi];
                                the two heads' K=64 matmuls run CONCURRENTLY
                                in distinct PE row groups)
  E_T = exp(S_T / sqrt(dk))    (ScalarE; pair tile exp'd in ONE merged N=2048
                                ACTIVATE + a solo tile -> amortizes the fixed
                                per-instruction cost without cross-WARs)
  E_T *= maskT (keep 0/1)      (DVE tensor_tensor, bf16 2x mode; merged over
                                3 strips [128,3072] via a 4D mask AP when the
                                strips share (hp,qb))
  O_T[dv', qi] += V'[kj]^T E_T (PE accumulate over kj)
Epilogue: the unnormalized O_T[65, 512] (64 value rows + denominator row) is
copied PSUM->SBUF (DVE) and DMA'd out as-is; the softmax division and the
[dv, qi] -> [qi, dv] transpose happen on the HOST. This removes all PE
transposes and DVE reciprocal/multiply work from the device.

The QK/exp emission runs LAG groups ahead of the mask/PV/epilogue phase so
the PE queue always has the next QK pair in front of PV work that waits on
the DVE.
"""

import math

import numpy as np

import concourse.bass as bass
import concourse.mybir as mybir
import concourse.tile as tile
from concourse import bacc

F32 = mybir.dt.float32
BF16 = mybir.dt.bfloat16
AF = mybir.ActivationFunctionType
ALU = mybir.AluOpType

N_CORES = 8


def build_attention_nc(nheads: int, S: int, DK: int, scale: float) -> bass.Bass:
    nc = bacc.Bacc("TRN2", target_bir_lowering=False, debug=False,
                   num_devices=N_CORES)

    DV1 = DK + 1          # V plus a ones column for softmax denominators
    n_kj = S // 128       # kj tiles per head
    QBLK = 512            # qi span of one O_T accumulator
    n_qblk = S // QBLK
    CH = S // 128         # 128-row chunks along seq
    NCHK = 4              # q/k S-chunks per head-pair
    CHK = S // NCHK       # columns per chunk (512)
    npairs = nheads // 2
    assert nheads % 2 == 0

    qt_d = nc.dram_tensor("qT", [npairs, NCHK, 128, CHK], BF16,
                          kind="ExternalInput")
    kt_d = nc.dram_tensor("kT", [npairs, NCHK, 128, CHK], BF16,
                          kind="ExternalInput")
    v1_d = nc.dram_tensor("v1", [nheads, 128, CH, DV1], BF16,
                          kind="ExternalInput")
    NMG = 4               # kt-groups for mask tiles
    m_d = nc.dram_tensor("maskT", [n_qblk, NMG, 128, n_kj // NMG, QBLK], BF16,
                         kind="ExternalInput")
    o_d = nc.dram_tensor("out", [nheads, n_qblk, DV1, QBLK], F32,
                         kind="ExternalOutput")

    with tile.TileContext(nc) as tc:
        with (
            tc.tile_pool(name="maskp", bufs=1) as maskp,
            tc.tile_pool(name="qkT", bufs=3) as qkt,
            tc.tile_pool(name="vp", bufs=3) as vp,
            tc.tile_pool(name="ep", bufs=12) as ep,
            tc.tile_pool(name="outp", bufs=4) as outp,
            tc.tile_pool(name="ring", bufs=1, space="PSUM") as ringp,
            tc.tile_pool(name="opsum", bufs=2, space="PSUM") as opsum,
        ):
            # ---- per-pair inputs: plain chunked DMAs.
            qk_t = {}     # hp -> (q chunk tiles, k chunk tiles)
            v1s_all = {}  # hp -> [v1_h0, v1_h1]

            def emit_pair_loads(hp, eng, veng):
                kts, qts = [], []
                # k chunk 0 + q chunk 0 first: they unblock the first QKs.
                for c in range(NCHK):
                    kc = qkt.tile([128, CHK], BF16, tag=f"ktc{c}",
                                  name=f"ktc{c}_{hp}")
                    eng.dma_start(out=kc, in_=kt_d[hp, c])
                    kts.append(kc)
                    qc = qkt.tile([128, CHK], BF16, tag=f"qtc{c}",
                                  name=f"qtc{c}_{hp}")
                    eng.dma_start(out=qc, in_=qt_d[hp, c])
                    qts.append(qc)
                qk_t[hp] = (qts, kts)
                v1s = []
                for i in (0, 1):
                    v1 = vp.tile([128, CH, DV1], BF16, tag=f"v1_{i}",
                                 name=f"v1_{2 * hp + i}")
                    veng.dma_start(out=v1, in_=v1_d[2 * hp + i])
                    v1s.append(v1)
                v1s_all[hp] = v1s

            # Prologue DMA schedule, ordered by need-time. gpsimd starts
            # fastest: it gets the two chunks the first QK strips need, then
            # the first mask tiles and v1. The mask lives in sixteen
            # [128, 4kt, 512qi] tiles (one contiguous 512KB DMA each, keyed
            # (kt-group, qb)) so each PV strip only gates on the 512KB tile
            # it actually reads -- a single DMA stream moves ~100GB/s, so
            # fine need-aligned granularity beats big transfers.
            MQ = n_kj // NMG            # kt per mask tile (4)
            kts0, qts0 = [], []
            for c in range(NCHK):
                kts0.append(qkt.tile([128, CHK], BF16, tag=f"ktc{c}",
                                     name=f"ktc{c}_0"))
                qts0.append(qkt.tile([128, CHK], BF16, tag=f"qtc{c}",
                                     name=f"qtc{c}_0"))
            qk_t[0] = (qts0, kts0)

            mtiles = {}
            for qb in range(n_qblk):
                for g in range(NMG):
                    mtiles[(g, qb)] = maskp.tile(
                        [128, MQ, QBLK], BF16, tag=f"m{g}_{qb}",
                        name=f"m{g}_{qb}")

            def mload(eng, g, qb):
                eng.dma_start(out=mtiles[(g, qb)], in_=m_d[qb, g])

            # tiny first chunk: the very first QK strip only needs kT cols
            # 0-127, so a 32KB DMA unblocks it ~1us before the full chunk
            kc0a = qkt.tile([128, 128], BF16, tag="ktc0a", name="ktc0a")
            nc.gpsimd.dma_start(out=kc0a, in_=kt_d[0, 0, :, 0:128])
            nc.gpsimd.dma_start(out=qts0[0], in_=qt_d[0, 0])
            nc.sync.dma_start(out=kts0[0], in_=kt_d[0, 0])
            nc.sync.dma_start(out=kts0[1], in_=kt_d[0, 1])
            nc.sync.dma_start(out=kts0[2], in_=kt_d[0, 2])
            mload(nc.gpsimd, 0, 0)
            v1s0 = []
            for i in (0, 1):
                v1 = vp.tile([128, CH, DV1], BF16, tag=f"v1_{i}",
                             name=f"v1_{i}")
                nc.gpsimd.dma_start(out=v1, in_=v1_d[i])
                v1s0.append(v1)
            v1s_all[0] = v1s0
            nc.sync.dma_start(out=kts0[3], in_=kt_d[0, 3])
            mload(nc.sync, 1, 0)
            mload(nc.gpsimd, 2, 0)
            nc.sync.dma_start(out=qts0[1], in_=qt_d[0, 1])
            mload(nc.sync, 3, 0)
            mload(nc.gpsimd, 0, 1)
            mload(nc.gpsimd, 1, 1)
            mload(nc.sync, 2, 1)
            mload(nc.sync, 3, 1)
            nc.sync.dma_start(out=qts0[2], in_=qt_d[0, 2])
            nc.sync.dma_start(out=qts0[3], in_=qt_d[0, 3])
            for qb in (2, 3):
                for g in range(NMG):
                    mload(nc.gpsimd if (g + qb) % 2 else nc.sync, g, qb)
            if npairs > 1:
                emit_pair_loads(1, nc.sync, nc.gpsimd)

            # ---- PSUM layout -----------------------------------------------
            # pairt: 2 strip slots for the merged-exp pairs (4 banks),
            # solot: 1 slot (2 banks) -> their WARs stay independent;
            # opsum: ps_o tiles share one rotating 2-buf tag (2 banks).
            pairt = ringp.tile([128, 2, 2 * QBLK], F32, tag="pair",
                               name="pairt")
            solot = ringp.tile([128, 2 * QBLK], F32, tag="solo", name="solot")

            # Dummy activation on scratch data: forces the ACT function-table
            # load + bias-AP setup to happen during the DMA prologue instead
            # of delaying the first real exp.
            warm = ep.tile([128, 8], BF16, tag="warm", name="warm")
            nc.vector.memset(warm, 0.0)
            nc.scalar.activation(warm, warm, AF.Exp, scale=scale)

            # ---- main loop --------------------------------------------------
            n_strips = npairs * n_qblk * n_kj

            def strip_info(s):
                hp = s // (n_qblk * n_kj)
                qb = (s // n_kj) % n_qblk
                kj = s % n_kj
                return hp, qb, kj

            ps_o = {}     # (hp, qb) -> [ps_o_h0, ps_o_h1]
            e_of = {}     # s -> (e3_tile, slot)

            def emit_qk(s):
                hp, qb, kj = strip_info(s)
                qts, kts = qk_t[hp]
                slot = s % 3
                dst = pairt[:, slot, :] if slot < 2 else solot
                if s == 0:
                    kc, k0 = kc0a, 0
                else:
                    kc = kts[kj // (n_kj // NCHK)]
                    k0 = (kj % (n_kj // NCHK)) * 128
                qc = qts[qb * QBLK // CHK]
                q0 = (qb * QBLK) % CHK
                for i in (0, 1):
                    nc.tensor.matmul(
                        dst[:, i * QBLK : (i + 1) * QBLK],
                        kc[64 * i : 64 * i + DK, k0 : k0 + 128],
                        qc[64 * i : 64 * i + DK, q0 : q0 + QBLK],
                        start=True, stop=True,
                    )

            def get_e3(s):
                """e3 tile shared by the 3 strips of s's triple."""
                t0 = (s // 3) * 3
                if t0 not in e_of:
                    e_of[t0] = ep.tile([128, 3, 2 * QBLK], BF16, tag="e3",
                                       name=f"e3_{t0}")
                return e_of[t0]

            def emit_exp_pair(s):
                # strips s (slot 0) and s+1 (slot 1) in one N=2048 ACTIVATE
                e3 = get_e3(s)
                nc.scalar.activation(e3[:, 0:2, :], pairt, AF.Exp, scale=scale)

            def emit_exp_solo(s):
                e3 = get_e3(s)
                nc.scalar.activation(e3[:, 2, :], solot, AF.Exp, scale=scale)

            def emit_exp_tail(s):
                # final unpaired strip landed on a pair slot
                e3 = get_e3(s)
                nc.scalar.activation(e3[:, s % 3, :], pairt[:, s % 3, :],
                                     AF.Exp, scale=scale)

            def emit_mask_strip(s):
                """fallback: mask one strip [128, 1024] with dup'd mask."""
                hp, qb, kj = strip_info(s)
                e3 = get_e3(s)
                ev = e3[:, s % 3, :]
                msl = mtiles[(kj // MQ, qb)][:, kj % MQ, :]
                mdup = bass.AP(
                    tensor=msl.tensor, offset=msl.offset,
                    ap=[msl.ap[0], [0, 2], [1, QBLK]],
                )
                nc.vector.tensor_mul(ev, ev, mdup)

            def emit_mask_triple(s0):
                """merged: mask strips s0..s0+2 in one [128, 3072] DVE op."""
                hp, qb, kj = strip_info(s0)
                e3 = e_of[s0]
                msl = mtiles[(kj // MQ, qb)][:, kj % MQ, :]
                m4 = bass.AP(
                    tensor=msl.tensor, offset=msl.offset,
                    ap=[msl.ap[0], [QBLK, 3], [0, 2], [1, QBLK]],
                )
                nc.vector.tensor_mul(e3, e3, m4)

            due_drains = []   # blocks whose ps_o awaits its PSUM->SBUF drain

            def emit_pv(s):
                hp, qb, kj = strip_info(s)
                e3 = e_of[(s // 3) * 3]
                last = kj == n_kj - 1
                for i in (0, 1):
                    nc.tensor.matmul(
                        ps_o[(hp, qb)][i],
                        v1s_all[hp][i][:, kj, :],
                        e3[:, s % 3, i * QBLK : (i + 1) * QBLK],
                        start=(kj == 0), stop=last,
                        skip_group_check=True,
                    )
                if last:
                    due_drains.append((hp, qb))

            def flush_drains():
                # The drain copies are emitted AFTER the next triple's mask
                # so they sit behind it in the in-order DVE queue: the mask
                # (which the next block's first PV needs) is not stuck behind
                # copies that wait on the epilogue PV.
                while due_drains:
                    hp, qb = due_drains.pop(0)
                    for i in (0, 1):
                        h = 2 * hp + i
                        ot_sb = outp.tile([DV1, QBLK], F32, tag="ot",
                                          name=f"ot_{h}_{qb}")
                        nc.vector.tensor_copy(ot_sb, ps_o[(hp, qb)][i])
                        nc.gpsimd.dma_start(out=o_d[h, qb], in_=ot_sb)
                    del ps_o[(hp, qb)]

            def ensure_ps_o(s):
                hp, qb, kj = strip_info(s)
                if kj == 0:
                    ps_o[(hp, qb)] = [
                        opsum.tile([DV1, QBLK], F32, tag="o",
                                   name=f"ps_o_{hp}_{qb}_{i}")
                        for i in (0, 1)
                    ]

            def post_triple(strips):
                """mask + PV + epilogue for a triple of strips."""
                s0 = strips[0]
                kj0 = s0 % n_kj
                merged = (
                    len(strips) == 3
                    and kj0 <= n_kj - 3
                    and kj0 // MQ == (kj0 + 2) // MQ
                )
                if merged:
                    emit_mask_triple(s0)
                else:
                    for t in strips:
                        emit_mask_strip(t)
                flush_drains()
                for t in strips:
                    hp, qb, kj = strip_info(t)
                    ensure_ps_o(t)
                    emit_pv(t)
                    # prefetch two pairs ahead early in qb0 (pairs 0/1 are
                    # loaded in the prologue)
                    if hp + 2 < npairs and qb == 0 and kj == 2:
                        emit_pair_loads(hp + 2, nc.sync, nc.gpsimd)

            # group strips by psum slot: slots (0,1) -> merged exp, slot 2 ->
            # solo. QK+exp emission runs LAG groups ahead of mask/PV/epilogue
            # so the PE queue always has the next QK pair in front of PV work
            # that waits on the DVE.
            groups = []
            s = 0
            while s < n_strips:
                if s % 3 == 0 and s + 1 < n_strips:
                    groups.append((s, s + 1))
                    s += 2
                else:
                    groups.append((s,))
                    s += 1

            # LAG control: a deep lag at startup keeps PV (which waits on the
            # mask DMAs) out of the in-order PE queue until the mask has
            # landed; at qb-block boundaries the first-kj PV waits on the ps_o
            # WAR (DVE copies), so those triples get extra slack too.
            LAG = 4
            START_THR = 11
            START_UNTIL = 24
            BOUND_EXTRA = 3
            COOLDOWN = 6
            pending = []
            postq = []
            cooldown = 0

            def next_post_strip():
                if postq:
                    return postq[0]
                if pending:
                    return pending[0][0]
                return None

            def want_thr():
                s0 = next_post_strip()
                if s0 is None:
                    return LAG
                if s0 < START_UNTIL:
                    return START_THR
                # next triple contains a kj==0 strip (its PV waits the ps_o
                # WAR on the previous block's DVE drain) -> extra slack
                if s0 % n_kj >= n_kj - 2 or s0 % n_kj == 0:
                    return LAG + BOUND_EXTRA
                return LAG

            def post_ready(force=False):
                """Post queued strips. Triples that straddle a qb-block
                boundary are posted strip-by-strip with a cooldown before the
                kj==0 strip, so QK subgroups land between the epilogue drain
                and the next block's first PV in the in-order PE queue."""
                nonlocal postq, cooldown
                while postq:
                    t0 = (postq[0] // 3) * 3
                    crossing = t0 % n_kj >= n_kj - 2
                    if postq[0] % n_kj == 0 and cooldown > 0 and not force:
                        break
                    if crossing:
                        s0 = postq.pop(0)
                        post_triple([s0])
                        if s0 % n_kj == n_kj - 1:
                            cooldown = COOLDOWN + 1
                    elif len(postq) >= 3 or force:
                        take, postq = postq[:3], postq[3:]
                        post_triple(take)
                        if any(t % n_kj == n_kj - 1 for t in take):
                            cooldown = COOLDOWN + 1
                    else:
                        break

            for g in groups:
                for t in g:
                    emit_qk(t)
                if len(g) == 2:
                    emit_exp_pair(g[0])
                elif g[0] % 3 == 2:
                    emit_exp_solo(g[0])
                else:
                    emit_exp_tail(g[0])
                pending.append(g)
                if cooldown > 0:
                    cooldown -= 1
                post_ready()
                while len(pending) > want_thr():
                    postq.extend(pending.pop(0))
                    post_ready()
            while pending:
                postq.extend(pending.pop(0))
            post_ready(force=True)
            flush_drains()

    nc.compile()
    return nc


_NC_CACHE: dict = {}


def _get_nc(nheads, S, DK, scale):
    key = (nheads, S, DK, scale)
    if key not in _NC_CACHE:
        _NC_CACHE[key] = build_attention_nc(nheads, S, DK, scale)
    return _NC_CACHE[key]


def make_in_maps(queries, keys, values, d_k, mask):
    """Host-side sharding + layout prep. Returns (in_maps, shape_info)."""
    import ml_dtypes

    BF = ml_dtypes.bfloat16
    B, H, S, DK = queries.shape
    BH = B * H
    assert BH % N_CORES == 0
    hpc = BH // N_CORES
    npairs = hpc // 2
    CH = S // 128
    n_kj = S // 128
    NCHK = 4

    q = np.ascontiguousarray(queries.reshape(BH, S, DK)).astype(BF)
    k = np.ascontiguousarray(keys.reshape(BH, S, DK)).astype(BF)
    v = np.ascontiguousarray(values.reshape(BH, S, DK)).astype(BF)

    # qT/kT: [BH//2 pairs, NCHK, 128, S/NCHK] with head i of a pair on
    # partitions 64i..64i+63, DK-major, chunked along S.
    def to_pairT(x):
        # [BH, S, DK] -> [BH, DK, S] -> [BH//2, 2*DK, S] -> chunked
        xt = x.transpose(0, 2, 1)
        xt = xt.reshape(BH // 2, 2 * DK, NCHK, S // NCHK)
        return np.ascontiguousarray(xt.transpose(0, 2, 1, 3))

    qT = to_pairT(q)
    kT = to_pairT(k)

    # v1: [BH, 128, CH, DK+1] with ones column baked in.
    v1 = np.ones((BH, 128, CH, DK + 1), dtype=BF)
    v1[:, :, :, :DK] = v.reshape(BH, CH, 128, DK).transpose(0, 2, 1, 3)

    # maskT: [n_qblk, 4, 128, 4, 512] bf16 keep-mask (1 - mask), tiled per
    # (qb-block, kt-group) so each tile is one contiguous 512KB DMA.
    NMG = 4
    QBLK = 512
    n_qblk = S // QBLK
    mT = (1 - mask.reshape(S, S)).astype(BF).T  # [kj, qi]
    mT = mT.reshape(NMG, n_kj // NMG, 128, n_qblk, QBLK)  # [g, j, p, qb, qi']
    mT = np.ascontiguousarray(mT.transpose(3, 0, 2, 1, 4))

    in_maps = [
        {
            "qT": qT[c * npairs : (c + 1) * npairs],
            "kT": kT[c * npairs : (c + 1) * npairs],
            "v1": v1[c * hpc : (c + 1) * hpc],
            "maskT": mT,
        }
        for c in range(N_CORES)
    ]
    return in_maps, (B, H, S, DK, hpc)


def kernel(queries, keys, values, d_k, mask):
    from concourse.bass_utils import run_bass_kernel_spmd

    in_maps, (B, H, S, DK, hpc) = make_in_maps(queries, keys, values, d_k,
                                               mask)
    scale = 1.0 / math.sqrt(float(d_k))
    nc = _get_nc(hpc, S, DK, scale)

    res = run_bass_kernel_spmd(nc, in_maps, core_ids=list(range(N_CORES)))
    outs = []
    for r in res.results:
        O = np.asarray(r["out"])            # [hpc, n_qblk, DK+1, QBLK]
        num = O[:, :, :DK, :]
        den = O[:, :, DK : DK + 1, :]
        o = (num / den).transpose(0, 1, 3, 2)   # [hpc, n_qblk, QBLK, DK]
        outs.append(o.reshape(hpc, S, DK))
    out = np.concatenate(outs, axis=0)
    return out.reshape(B, H, S, DK).astype(queries.dtype)


# revision 40
# speedup vs baseline: 1.0029x; 1.0029x over previous
"""Masked dot-product attention on 8 Trainium2 NeuronCores.

Strategy (per core): head-parallel sharding. B*H = 64 (batch, head) pairs are
split 8 per core; each core runs the full attention for its heads.

All layout transforms happen on the HOST (numpy) so the device only issues
plain contiguous DMAs:
  qT/kT:  [npairs, 4, 128, 512] bf16, head i of a pair on partitions
          64i..64i+63, DK-major, chunked along S so compute can start as soon
          as the first 128KB chunk lands.
  v1:     [nheads, 128, CH, 65] bf16, kj-within-chunk on partitions, with the
          ones column baked in (row dv=64 accumulates softmax denominators).
  maskT:  [128, n_kj, S] bf16 keep-mask (1-mask), kj-within-tile on
          partitions.

Per-head-pair pipeline (S=2048, DK=64), "S-transposed" layout so the PV
matmul needs no transpose of the huge exp matrix:
  S_T[kj, qi] = K @ Q^T        (PE, bf16, psum strips [128 kj, 2x512 qi];
                                the two heads' K=64 matmuls run CONCURRENTLY
                                in distinct PE row groups)
  E_T = exp(S_T / sqrt(dk))    (ScalarE; pair tile exp'd in ONE merged N=2048
                                ACTIVATE + a solo tile -> amortizes the fixed
                                per-instruction cost without cross-WARs)
  E_T *= maskT (keep 0/1)      (DVE tensor_tensor, bf16 2x mode; merged over
                                3 strips [128,3072] via a 4D mask AP when the
                                strips share (hp,qb))
  O_T[dv', qi] += V'[kj]^T E_T (PE accumulate over kj)
Epilogue: the unnormalized O_T[65, 512] (64 value rows + denominator row) is
copied PSUM->SBUF (DVE) and DMA'd out as-is; the softmax division and the
[dv, qi] -> [qi, dv] transpose happen on the HOST. This removes all PE
transposes and DVE reciprocal/multiply work from the device.

The QK/exp emission runs LAG groups ahead of the mask/PV/epilogue phase so
the PE queue always has the next QK pair in front of PV work that waits on
the DVE.
"""

import math

import numpy as np

import concourse.bass as bass
import concourse.mybir as mybir
import concourse.tile as tile
from concourse import bacc

F32 = mybir.dt.float32
BF16 = mybir.dt.bfloat16
AF = mybir.ActivationFunctionType
ALU = mybir.AluOpType

N_CORES = 8


def build_attention_nc(nheads: int, S: int, DK: int, scale: float) -> bass.Bass:
    nc = bacc.Bacc("TRN2", target_bir_lowering=False, debug=False,
                   num_devices=N_CORES)

    DV1 = DK + 1          # V plus a ones column for softmax denominators
    n_kj = S // 128       # kj tiles per head
    QBLK = 512            # qi span of one O_T accumulator
    n_qblk = S // QBLK
    CH = S // 128         # 128-row chunks along seq
    NCHK = 4              # q/k S-chunks per head-pair
    CHK = S // NCHK       # columns per chunk (512)
    npairs = nheads // 2
    assert nheads % 2 == 0

    qt_d = nc.dram_tensor("qT", [npairs, NCHK, 128, CHK], BF16,
                          kind="ExternalInput")
    kt_d = nc.dram_tensor("kT", [npairs, NCHK, 128, CHK], BF16,
                          kind="ExternalInput")
    v1_d = nc.dram_tensor("v1", [nheads, 128, CH, DV1], BF16,
                          kind="ExternalInput")
    NMG = 4               # kt-groups for mask tiles
    m_d = nc.dram_tensor("maskT", [n_qblk, NMG, 128, n_kj // NMG, QBLK], BF16,
                         kind="ExternalInput")
    o_d = nc.dram_tensor("out", [nheads, n_qblk, DV1, QBLK], F32,
                         kind="ExternalOutput")

    with tile.TileContext(nc) as tc:
        with (
            tc.tile_pool(name="maskp", bufs=1) as maskp,
            tc.tile_pool(name="qkT", bufs=3) as qkt,
            tc.tile_pool(name="vp", bufs=3) as vp,
            tc.tile_pool(name="ep", bufs=12) as ep,
            tc.tile_pool(name="outp", bufs=4) as outp,
            tc.tile_pool(name="ring", bufs=1, space="PSUM") as ringp,
            tc.tile_pool(name="opsum", bufs=2, space="PSUM") as opsum,
        ):
            # ---- per-pair inputs: plain chunked DMAs.
            qk_t = {}     # hp -> (q chunk tiles, k chunk tiles)
            v1s_all = {}  # hp -> [v1_h0, v1_h1]

            def emit_pair_loads(hp, eng, veng):
                kts, qts = [], []
                # k chunk 0 + q chunk 0 first: they unblock the first QKs.
                for c in range(NCHK):
                    kc = qkt.tile([128, CHK], BF16, tag=f"ktc{c}",
                                  name=f"ktc{c}_{hp}")
                    eng.dma_start(out=kc, in_=kt_d[hp, c])
                    kts.append(kc)
                    qc = qkt.tile([128, CHK], BF16, tag=f"qtc{c}",
                                  name=f"qtc{c}_{hp}")
                    eng.dma_start(out=qc, in_=qt_d[hp, c])
                    qts.append(qc)
                qk_t[hp] = (qts, kts)
                v1s = []
                for i in (0, 1):
                    v1 = vp.tile([128, CH, DV1], BF16, tag=f"v1_{i}",
                                 name=f"v1_{2 * hp + i}")
                    veng.dma_start(out=v1, in_=v1_d[2 * hp + i])
                    v1s.append(v1)
                v1s_all[hp] = v1s

            # Prologue DMA schedule, ordered by need-time. gpsimd starts
            # fastest: it gets the two chunks the first QK strips need, then
            # the first mask tiles and v1. The mask lives in sixteen
            # [128, 4kt, 512qi] tiles (one contiguous 512KB DMA each, keyed
            # (kt-group, qb)) so each PV strip only gates on the 512KB tile
            # it actually reads -- a single DMA stream moves ~100GB/s, so
            # fine need-aligned granularity beats big transfers.
            MQ = n_kj // NMG            # kt per mask tile (4)
            kts0, qts0 = [], []
            for c in range(NCHK):
                kts0.append(qkt.tile([128, CHK], BF16, tag=f"ktc{c}",
                                     name=f"ktc{c}_0"))
                qts0.append(qkt.tile([128, CHK], BF16, tag=f"qtc{c}",
                                     name=f"qtc{c}_0"))
            qk_t[0] = (qts0, kts0)

            mtiles = {}
            for qb in range(n_qblk):
                for g in range(NMG):
                    mtiles[(g, qb)] = maskp.tile(
                        [128, MQ, QBLK], BF16, tag=f"m{g}_{qb}",
                        name=f"m{g}_{qb}")

            def mload(eng, g, qb):
                eng.dma_start(out=mtiles[(g, qb)], in_=m_d[qb, g])

            # tiny first chunk: the very first QK strip only needs kT cols
            # 0-127, so a 32KB DMA unblocks it ~1us before the full chunk
            kc0a = qkt.tile([128, 128], BF16, tag="ktc0a", name="ktc0a")
            nc.gpsimd.dma_start(out=kc0a, in_=kt_d[0, 0, :, 0:128])
            nc.gpsimd.dma_start(out=qts0[0], in_=qt_d[0, 0])
            nc.sync.dma_start(out=kts0[0], in_=kt_d[0, 0])
            nc.sync.dma_start(out=kts0[1], in_=kt_d[0, 1])
            nc.sync.dma_start(out=kts0[2], in_=kt_d[0, 2])
            mload(nc.gpsimd, 0, 0)
            v1s0 = []
            for i in (0, 1):
                v1 = vp.tile([128, CH, DV1], BF16, tag=f"v1_{i}",
                             name=f"v1_{i}")
                nc.gpsimd.dma_start(out=v1, in_=v1_d[i])
                v1s0.append(v1)
            v1s_all[0] = v1s0
            nc.sync.dma_start(out=kts0[3], in_=kt_d[0, 3])
            mload(nc.sync, 1, 0)
            mload(nc.gpsimd, 2, 0)
            nc.sync.dma_start(out=qts0[1], in_=qt_d[0, 1])
            mload(nc.sync, 3, 0)
            mload(nc.gpsimd, 0, 1)
            mload(nc.gpsimd, 1, 1)
            mload(nc.sync, 2, 1)
            mload(nc.sync, 3, 1)
            nc.sync.dma_start(out=qts0[2], in_=qt_d[0, 2])
            nc.sync.dma_start(out=qts0[3], in_=qt_d[0, 3])
            for qb in (2, 3):
                for g in range(NMG):
                    mload(nc.gpsimd if (g + qb) % 2 else nc.sync, g, qb)
            if npairs > 1:
                emit_pair_loads(1, nc.sync, nc.gpsimd)

            # ---- PSUM layout -----------------------------------------------
            # pairt: 2 strip slots for the merged-exp pairs (4 banks),
            # solot: 1 slot (2 banks) -> their WARs stay independent;
            # opsum: ps_o tiles share one rotating 2-buf tag (2 banks).
            pairt = ringp.tile([128, 2, 2 * QBLK], F32, tag="pair",
                               name="pairt")
            solot = ringp.tile([128, 2 * QBLK], F32, tag="solo", name="solot")

            # Dummy activation on scratch data: forces the ACT function-table
            # load + bias-AP setup to happen during the DMA prologue instead
            # of delaying the first real exp.
            warm = ep.tile([128, 8], BF16, tag="warm", name="warm")
            nc.vector.memset(warm, 0.0)
            nc.scalar.activation(warm, warm, AF.Exp, scale=scale)

            # ---- main loop --------------------------------------------------
            n_strips = npairs * n_qblk * n_kj

            def strip_info(s):
                hp = s // (n_qblk * n_kj)
                qb = (s // n_kj) % n_qblk
                kj = s % n_kj
                return hp, qb, kj

            ps_o = {}     # (hp, qb) -> [ps_o_h0, ps_o_h1]
            e_of = {}     # s -> (e3_tile, slot)

            def emit_qk(s):
                hp, qb, kj = strip_info(s)
                qts, kts = qk_t[hp]
                slot = s % 3
                dst = pairt[:, slot, :] if slot < 2 else solot
                if s == 0:
                    kc, k0 = kc0a, 0
                else:
                    kc = kts[kj // (n_kj // NCHK)]
                    k0 = (kj % (n_kj // NCHK)) * 128
                qc = qts[qb * QBLK // CHK]
                q0 = (qb * QBLK) % CHK
                for i in (0, 1):
                    nc.tensor.matmul(
                        dst[:, i * QBLK : (i + 1) * QBLK],
                        kc[64 * i : 64 * i + DK, k0 : k0 + 128],
                        qc[64 * i : 64 * i + DK, q0 : q0 + QBLK],
                        start=True, stop=True,
                    )

            def get_e3(s):
                """e3 tile shared by the 3 strips of s's triple."""
                t0 = (s // 3) * 3
                if t0 not in e_of:
                    e_of[t0] = ep.tile([128, 3, 2 * QBLK], BF16, tag="e3",
                                       name=f"e3_{t0}")
                return e_of[t0]

            def emit_exp_pair(s):
                # strips s (slot 0) and s+1 (slot 1) in one N=2048 ACTIVATE
                e3 = get_e3(s)
                nc.scalar.activation(e3[:, 0:2, :], pairt, AF.Exp, scale=scale)

            def emit_exp_solo(s):
                e3 = get_e3(s)
                nc.scalar.activation(e3[:, 2, :], solot, AF.Exp, scale=scale)

            def emit_exp_tail(s):
                # final unpaired strip landed on a pair slot
                e3 = get_e3(s)
                nc.scalar.activation(e3[:, s % 3, :], pairt[:, s % 3, :],
                                     AF.Exp, scale=scale)

            def emit_mask_strip(s):
                """fallback: mask one strip [128, 1024] with dup'd mask."""
                hp, qb, kj = strip_info(s)
                e3 = get_e3(s)
                ev = e3[:, s % 3, :]
                msl = mtiles[(kj // MQ, qb)][:, kj % MQ, :]
                mdup = bass.AP(
                    tensor=msl.tensor, offset=msl.offset,
                    ap=[msl.ap[0], [0, 2], [1, QBLK]],
                )
                nc.vector.tensor_mul(ev, ev, mdup)

            def emit_mask_triple(s0):
                """merged: mask strips s0..s0+2 in one [128, 3072] DVE op."""
                hp, qb, kj = strip_info(s0)
                e3 = e_of[s0]
                msl = mtiles[(kj // MQ, qb)][:, kj % MQ, :]
                m4 = bass.AP(
                    tensor=msl.tensor, offset=msl.offset,
                    ap=[msl.ap[0], [QBLK, 3], [0, 2], [1, QBLK]],
                )
                nc.vector.tensor_mul(e3, e3, m4)

            due_drains = []   # blocks whose ps_o awaits its PSUM->SBUF drain

            def emit_pv(s):
                hp, qb, kj = strip_info(s)
                e3 = e_of[(s // 3) * 3]
                last = kj == n_kj - 1
                for i in (0, 1):
                    nc.tensor.matmul(
                        ps_o[(hp, qb)][i],
                        v1s_all[hp][i][:, kj, :],
                        e3[:, s % 3, i * QBLK : (i + 1) * QBLK],
                        start=(kj == 0), stop=last,
                        skip_group_check=True,
                    )
                if last:
                    due_drains.append((hp, qb))

            def flush_drains():
                # The drain copies are emitted AFTER the next triple's mask
                # so they sit behind it in the in-order DVE queue: the mask
                # (which the next block's first PV needs) is not stuck behind
                # copies that wait on the epilogue PV.
                while due_drains:
                    hp, qb = due_drains.pop(0)
                    for i in (0, 1):
                        h = 2 * hp + i
                        ot_sb = outp.tile([DV1, QBLK], F32, tag="ot",
                                          name=f"ot_{h}_{qb}")
                        nc.vector.tensor_copy(ot_sb, ps_o[(hp, qb)][i])
                        nc.gpsimd.dma_start(out=o_d[h, qb], in_=ot_sb)
                    del ps_o[(hp, qb)]

            def ensure_ps_o(s):
                hp, qb, kj = strip_info(s)
                if kj == 0:
                    ps_o[(hp, qb)] = [
                        opsum.tile([DV1, QBLK], F32, tag="o",
                                   name=f"ps_o_{hp}_{qb}_{i}")
                        for i in (0, 1)
                    ]

            def post_triple(strips):
                """mask + PV + epilogue for a triple of strips."""
                s0 = strips[0]
                kj0 = s0 % n_kj
                merged = (
                    len(strips) == 3
                    and kj0 <= n_kj - 3
                    and kj0 // MQ == (kj0 + 2) // MQ
                )
                if merged:
                    emit_mask_triple(s0)
                else:
                    for t in strips:
                        emit_mask_strip(t)
                flush_drains()
                for t in strips:
                    hp, qb, kj = strip_info(t)
                    ensure_ps_o(t)
                    emit_pv(t)
                    # prefetch two pairs ahead early in qb0 (pairs 0/1 are
                    # loaded in the prologue)
                    if hp + 2 < npairs and qb == 0 and kj == 2:
                        emit_pair_loads(hp + 2, nc.sync, nc.gpsimd)

            # group strips by psum slot: slots (0,1) -> merged exp, slot 2 ->
            # solo. QK+exp emission runs LAG groups ahead of mask/PV/epilogue
            # so the PE queue always has the next QK pair in front of PV work
            # that waits on the DVE.
            groups = []
            s = 0
            while s < n_strips:
                if s % 3 == 0 and s + 1 < n_strips:
                    groups.append((s, s + 1))
                    s += 2
                else:
                    groups.append((s,))
                    s += 1

            # LAG control: a deep lag at startup keeps PV (which waits on the
            # mask DMAs) out of the in-order PE queue until the mask has
            # landed; at qb-block boundaries the first-kj PV waits on the ps_o
            # WAR (DVE copies), so those triples get extra slack too.
            LAG = 3
            START_THR = 10
            START_UNTIL = 24
            BOUND_EXTRA = 3
            COOLDOWN = 5
            pending = []
            postq = []
            cooldown = 0

            def next_post_strip():
                if postq:
                    return postq[0]
                if pending:
                    return pending[0][0]
                return None

            def want_thr():
                s0 = next_post_strip()
                if s0 is None:
                    return LAG
                if s0 < START_UNTIL:
                    return START_THR
                # drain mode: shrink the lag over the last strips so the
                # post-work (mask/PV/drain) overlaps the final exps instead
                # of bunching up after them
                if s0 >= n_strips - 11:
                    return 1
                # next triple contains a kj==0 strip (its PV waits the ps_o
                # WAR on the previous block's DVE drain) -> extra slack
                if s0 % n_kj >= n_kj - 2 or s0 % n_kj == 0:
                    return LAG + BOUND_EXTRA
                return LAG

            def post_ready(force=False):
                """Post queued strips. Triples that straddle a qb-block
                boundary are posted strip-by-strip with a cooldown before the
                kj==0 strip, so QK subgroups land between the epilogue drain
                and the next block's first PV in the in-order PE queue."""
                nonlocal postq, cooldown
                while postq:
                    t0 = (postq[0] // 3) * 3
                    crossing = t0 % n_kj >= n_kj - 2
                    if postq[0] % n_kj == 0 and cooldown > 0 and not force:
                        break
                    if crossing:
                        s0 = postq.pop(0)
                        post_triple([s0])
                        if s0 % n_kj == n_kj - 1:
                            cooldown = COOLDOWN + 1
                    elif len(postq) >= 3 or force:
                        take, postq = postq[:3], postq[3:]
                        post_triple(take)
                        if any(t % n_kj == n_kj - 1 for t in take):
                            cooldown = COOLDOWN + 1
                    else:
                        break

            for g in groups:
                for t in g:
                    emit_qk(t)
                if len(g) == 2:
                    emit_exp_pair(g[0])
                elif g[0] % 3 == 2:
                    emit_exp_solo(g[0])
                else:
                    emit_exp_tail(g[0])
                pending.append(g)
                if cooldown > 0:
                    cooldown -= 1
                post_ready()
                while len(pending) > want_thr():
                    postq.extend(pending.pop(0))
                    post_ready()
            while pending:
                postq.extend(pending.pop(0))
            post_ready(force=True)
            flush_drains()

    nc.compile()
    return nc


_NC_CACHE: dict = {}


def _get_nc(nheads, S, DK, scale):
    key = (nheads, S, DK, scale)
    if key not in _NC_CACHE:
        _NC_CACHE[key] = build_attention_nc(nheads, S, DK, scale)
    return _NC_CACHE[key]


def make_in_maps(queries, keys, values, d_k, mask):
    """Host-side sharding + layout prep. Returns (in_maps, shape_info)."""
    import ml_dtypes

    BF = ml_dtypes.bfloat16
    B, H, S, DK = queries.shape
    BH = B * H
    assert BH % N_CORES == 0
    hpc = BH // N_CORES
    npairs = hpc // 2
    CH = S // 128
    n_kj = S // 128
    NCHK = 4

    q = np.ascontiguousarray(queries.reshape(BH, S, DK)).astype(BF)
    k = np.ascontiguousarray(keys.reshape(BH, S, DK)).astype(BF)
    v = np.ascontiguousarray(values.reshape(BH, S, DK)).astype(BF)

    # qT/kT: [BH//2 pairs, NCHK, 128, S/NCHK] with head i of a pair on
    # partitions 64i..64i+63, DK-major, chunked along S.
    def to_pairT(x):
        # [BH, S, DK] -> [BH, DK, S] -> [BH//2, 2*DK, S] -> chunked
        xt = x.transpose(0, 2, 1)
        xt = xt.reshape(BH // 2, 2 * DK, NCHK, S // NCHK)
        return np.ascontiguousarray(xt.transpose(0, 2, 1, 3))

    qT = to_pairT(q)
    kT = to_pairT(k)

    # v1: [BH, 128, CH, DK+1] with ones column baked in.
    v1 = np.ones((BH, 128, CH, DK + 1), dtype=BF)
    v1[:, :, :, :DK] = v.reshape(BH, CH, 128, DK).transpose(0, 2, 1, 3)

    # maskT: [n_qblk, 4, 128, 4, 512] bf16 keep-mask (1 - mask), tiled per
    # (qb-block, kt-group) so each tile is one contiguous 512KB DMA.
    NMG = 4
    QBLK = 512
    n_qblk = S // QBLK
    mT = (1 - mask.reshape(S, S)).astype(BF).T  # [kj, qi]
    mT = mT.reshape(NMG, n_kj // NMG, 128, n_qblk, QBLK)  # [g, j, p, qb, qi']
    mT = np.ascontiguousarray(mT.transpose(3, 0, 2, 1, 4))

    in_maps = [
        {
            "qT": qT[c * npairs : (c + 1) * npairs],
            "kT": kT[c * npairs : (c + 1) * npairs],
            "v1": v1[c * hpc : (c + 1) * hpc],
            "maskT": mT,
        }
        for c in range(N_CORES)
    ]
    return in_maps, (B, H, S, DK, hpc)


def kernel(queries, keys, values, d_k, mask):
    from concourse.bass_utils import run_bass_kernel_spmd

    in_maps, (B, H, S, DK, hpc) = make_in_maps(queries, keys, values, d_k,
                                               mask)
    scale = 1.0 / math.sqrt(float(d_k))
    nc = _get_nc(hpc, S, DK, scale)

    res = run_bass_kernel_spmd(nc, in_maps, core_ids=list(range(N_CORES)))
    outs = []
    for r in res.results:
        O = np.asarray(r["out"])            # [hpc, n_qblk, DK+1, QBLK]
        num = O[:, :, :DK, :]
        den = O[:, :, DK : DK + 1, :]
        o = (num / den).transpose(0, 1, 3, 2)   # [hpc, n_qblk, QBLK, DK]
        outs.append(o.reshape(hpc, S, DK))
    out = np.concatenate(outs, axis=0)
    return out.reshape(B, H, S, DK).astype(queries.dtype)
